# revision 27
# baseline (speedup 1.0000x reference)
"""Trainium2 Bass kernel for nn_MBSFeedForward (moe_routing) — fp8 edition.

Reference semantics (per token t with class c = type_seq[t]):
  c == 0:  out = LN_out(x_t)
  c >= 1:  e = c-1 (expert)
           u = GELU(x_t @ W1_e + b1_e) @ W2_e + b2_e
           v = LN_e(u + x_t)          (per-expert ln_g/ln_b)
           out = LN_out(v + x_t)      (out_g/out_b)

Sharding (host-side routing): 4 experts x 2 cores each; class-0 tokens
split over all 8 cores (outer LN only). No collectives; host scatters.

Device kernel design:
  - Both GEMMs run as fp8(e4m3) DoubleRow matmuls: each instruction
    contracts TWO 128-deep k-tiles (operands laid out [128, 2, n]) at
    0.5 PE cycles per output row. Host pre-quantizes x/W1/W2 with
    power-of-2 scales (RNE via ml_dtypes); the only on-device fp8
    rounding is the gelu->fp8 write, which the scalar engine does RNE
    (verified on HW). End-to-end rel err ~1.8e-2 vs the 2e-2 gate.
  - GEMM2 result arrives in PSUM scaled by sw2; LN is scale-invariant,
    so the first residual add computes sw2*(x+b2) + psU in one DVE
    scalar_tensor_tensor (sw2 rides along as a [P,1] scalar input) and
    the normalize needs no unscale. b2 folds into the residual array.
  - LayerNorm: bn_stats/bn_aggr on DVE, rsqrt via bit-trick + Newton on
    GPSIMD (keeps Sqrt off the scalar engine so the Gelu activation
    table is loaded exactly once), LN1 normalize on DVE (bf16 2x), the
    final normalize on GPSIMD writing f32.
  - All inputs are DMA'd to SBUF once, on the two HWDGE queues
    (SP/Activation) in the order chunk 0 consumes them; steady state
    does only output-store DMAs.
"""

import math
from contextlib import ExitStack

import numpy as np
import ml_dtypes

import concourse.bass as bass
import concourse.tile as tile
from concourse import bacc
from concourse import mybir
from concourse.bass_utils import run_bass_kernel_spmd

F32 = mybir.dt.float32
BF16 = mybir.dt.bfloat16
FP8 = mybir.dt.float8e4
I32 = mybir.dt.int32
E4M3 = ml_dtypes.float8_e4m3
NP_BF16 = ml_dtypes.bfloat16

P = 128
H = 768
F = 3072
KH = H // P     # 6  (k-tiles for GEMM1)
KF = F // P     # 24 (k-tiles for GEMM2)
TCH = 256       # tokens per chunk
TPT = TCH // P  # token tiles per chunk (2)
QK = 4          # kf-tiles per ps1/gelu batch
NQ = KF // QK   # 6 ps1 batches per chunk
NCORES = 8
RSQRT_MAGIC = 0x5F3759DF
FP8_TARGET = 192.0  # max|v*scale| target; e4m3 (IEEE) max finite is 240
NEWTON2 = True      # second Newton step for rsqrt

ACT_FUNC = mybir.ActivationFunctionType.Gelu


def _pow2_scale(m: float) -> float:
    if m <= 0 or not math.isfinite(m):
        return 1.0
    return 2.0 ** math.floor(math.log2(FP8_TARGET / m))


def build_nc(cap: int, cap0: int, repeat: int = 1, *, b1_zero: bool = True,
             b2_zero: bool = True, ln_trivial: bool = True,
             out_trivial: bool = True) -> bass.Bass:
    """Per-core Bass module for `cap` routed tokens and `cap0` LN-only
    tokens. repeat>1 re-runs the body (slope timing)."""
    assert cap % TCH == 0 and cap0 % P == 0
    nc = bacc.Bacc()
    chunks = cap // TCH
    nt0 = cap0 // P
    ntile = cap // P

    d_xrT = nc.dram_tensor("xrT", [H, cap], FP8, kind="ExternalInput")
    d_w1 = nc.dram_tensor("w1", [H, F], FP8, kind="ExternalInput")
    d_w2 = nc.dram_tensor("w2", [F, H], FP8, kind="ExternalInput")
    d_xr = nc.dram_tensor("xr", [cap, H], BF16, kind="ExternalInput")
    d_x0 = nc.dram_tensor("x0", [cap0, H], BF16, kind="ExternalInput")
    d_ginv1 = nc.dram_tensor("ginv1", [1], F32, kind="ExternalInput")
    d_s2 = nc.dram_tensor("s2", [1], F32, kind="ExternalInput")
    d_b1 = None if b1_zero else nc.dram_tensor("b1", [F], F32, kind="ExternalInput")
    # when b2 != 0, the residual-2 array (plain x) is separate from xr (x+b2)
    d_xr0 = None if b2_zero else nc.dram_tensor("xr0", [cap, H], BF16, kind="ExternalInput")
    d_lng = d_lnb = d_outg = d_outb = None
    if not ln_trivial:
        d_lng = nc.dram_tensor("lng", [H], F32, kind="ExternalInput")
        d_lnb = nc.dram_tensor("lnb", [H], F32, kind="ExternalInput")
    if not out_trivial:
        d_outg = nc.dram_tensor("outg", [H], F32, kind="ExternalInput")
        d_outb = nc.dram_tensor("outb", [H], F32, kind="ExternalInput")
    d_yr = nc.dram_tensor("yr", [cap, H], F32, kind="ExternalOutput")
    d_y0 = nc.dram_tensor("y0", [cap0, H], F32, kind="ExternalOutput")

    with tile.TileContext(nc) as tc, ExitStack() as ctx:
        singles = ctx.enter_context(tc.tile_pool(name="singles", bufs=1))
        hpool = ctx.enter_context(tc.tile_pool(name="hact", bufs=14))
        wbf = ctx.enter_context(tc.tile_pool(name="wbf", bufs=18))
        wf32 = ctx.enter_context(tc.tile_pool(name="wf32", bufs=8))
        stat = ctx.enter_context(tc.tile_pool(name="stat", bufs=48))
        ps1p = ctx.enter_context(tc.tile_pool(name="ps1", bufs=2, space="PSUM"))
        psUp = ctx.enter_context(tc.tile_pool(name="psU", bufs=2, space="PSUM"))

        # --- resident inputs on the two HWDGE queues, in the order chunk 0
        # consumes them: xrT c0, w1 (GEMM1), w2 (GEMM2), rest of xrT ---
        w1sb = singles.tile([P, KH, F], FP8)
        w1_v = d_w1[:].rearrange("(ko ki) f -> ki ko f", ki=P)
        w2sb = singles.tile([P, KF, H], FP8)
        w2_v = d_w2[:].rearrange("(ko ki) h -> ki ko h", ki=P)
        xrTsb = singles.tile([P, KH, cap], FP8)
        xrT_v = d_xrT[:].rearrange("(ko ki) t -> ki ko t", ki=P)
        c0 = min(3 * TCH, cap)
        nc.scalar.dma_start(xrTsb[:, :, 0:c0], xrT_v[:, :, 0:c0])
        # interleave w1 thirds with w2 sixths so chunk 0's GEMM2 operands
        # land while its GEMM1 still runs
        for s in range(3):
            sl = slice(s * F // 3, (s + 1) * F // 3)
            nc.scalar.dma_start(w1sb[:, :, sl], w1_v[:, :, sl])
            for t in (2 * s, 2 * s + 1):
                tl = slice(t * KF // 6, (t + 1) * KF // 6)
                nc.scalar.dma_start(w2sb[:, tl], w2_v[:, tl])
        if c0 < cap:
            nc.scalar.dma_start(xrTsb[:, :, c0:cap], xrT_v[:, :, c0:cap])

        x0sb = singles.tile([P, max(nt0, 1), H], BF16)
        if nt0:
            nc.sync.dma_start(x0sb[:, 0:nt0], d_x0[:].rearrange("(n p) h -> p n h", p=P))
        xrsb = singles.tile([P, ntile, H], BF16)
        xr_v = d_xr[:].rearrange("(n p) h -> p n h", p=P)
        nc.sync.dma_start(xrsb[:, 0:2], xr_v[:, 0:2])
        nc.sync.dma_start(xrsb[:, 2:ntile // 2], xr_v[:, 2:ntile // 2])
        nc.sync.dma_start(xrsb[:, ntile // 2:], xr_v[:, ntile // 2:])
        xr0sb = xrsb
        if not b2_zero:
            xr0sb = singles.tile([P, ntile, H], BF16, name="xr0sb")
            nc.sync.dma_start(xr0sb, d_xr0[:].rearrange("(n p) h -> p n h", p=P))

        ginv1 = singles.tile([P, 1], F32, name="ginv1")
        nc.gpsimd.dma_start(ginv1, d_ginv1[:][None, :].to_broadcast([P, 1]))
        s2t = singles.tile([P, 1], F32, name="s2t")
        nc.gpsimd.dma_start(s2t, d_s2[:][None, :].to_broadcast([P, 1]))

        def bc_tile(d, nm):
            t = singles.tile([P, H], F32, tag=nm, name=nm)
            nc.gpsimd.dma_start(t, d[:][None, :].to_broadcast([P, H]))
            return t

        lngbc = lnbbc = outgbc = outbbc = None
        if not ln_trivial:
            lngbc, lnbbc = bc_tile(d_lng, "lngbc"), bc_tile(d_lnb, "lnbbc")
        if not out_trivial:
            outgbc, outbbc = bc_tile(d_outg, "outgbc"), bc_tile(d_outb, "outbbc")
        b1sb = None
        if not b1_zero:
            b1sb = singles.tile([P, KF], F32, name="b1sb")
            nc.gpsimd.dma_start(b1sb, d_b1[:].rearrange("(o p) -> p o", p=P))

        def ln_stats(src, mv):
            """DVE: bn_stats+aggr of src [P, H] into mv ([P, 2] slice)."""
            st = stat.tile([P, 2, 6], F32, tag="st")
            nc.vector.bn_stats(st[:, 0], src[:, 0:384])
            nc.vector.bn_stats(st[:, 1], src[:, 384:768])
            nc.vector.bn_aggr(mv, st)

        def rsqrt_batch(v):
            """rs = 1/sqrt(v) elementwise for v [P, k] (k small). Quake seed
            y0 = float_bits(magic - (v_int >> 1)) built as (~(v>>1)) +
            (magic+1) — shift+xor fuse (bitwise pair) and there is no
            reversed subtract; GPSIMD fails the ISA check for shifts so the
            seed runs on DVE. Newton steps y' = (1.5 - 0.5*v*y^2)*y go on
            GPSIMD as stt/tt/stt with signs folded into the constants."""
            k = v.shape[-1]
            yi = stat.tile([P, k], I32, tag="yi")
            nc.vector.tensor_scalar(
                yi, v.bitcast(I32), scalar1=1, scalar2=-1,
                op0=mybir.AluOpType.logical_shift_right,
                op1=mybir.AluOpType.bitwise_xor)
            nc.vector.tensor_scalar(
                yi, yi, scalar1=RSQRT_MAGIC + 1, scalar2=None,
                op0=mybir.AluOpType.add)
            rs = yi.bitcast(F32)
            for _ in range(1 + int(NEWTON2)):
                z = stat.tile([P, k], F32, tag="z")
                nc.gpsimd.tensor_tensor(z, rs, rs, op=mybir.AluOpType.mult)
                nc.gpsimd.tensor_tensor(z, z, v, op=mybir.AluOpType.mult)
                nc.gpsimd.tensor_scalar(
                    z, z, scalar1=-0.5, scalar2=-1.5,
                    op0=mybir.AluOpType.mult, op1=mybir.AluOpType.subtract)
                rs_n = stat.tile([P, k], F32, tag="rsn")
                nc.gpsimd.tensor_tensor(rs_n, z, rs, op=mybir.AluOpType.mult)
                rs = rs_n
            return rs

        def normalize(src, dst, mv, rs, eng, gbc, bbc):
            eng.tensor_scalar(
                dst, src, scalar1=mv[:, 0:1], scalar2=rs,
                op0=mybir.AluOpType.subtract, op1=mybir.AluOpType.mult)
            if gbc is not None:
                nc.gpsimd.tensor_mul(dst, dst, gbc)
                nc.gpsimd.tensor_add(dst, dst, bbc)

        def neg_m_rs(mv, rs):
            """[P,k] bias tile -mean*rs for activation-engine normalizes:
            activation(Identity, scale=rs, bias=-m*rs) == (t - m) * rs."""
            k = rs.shape[-1]
            nrs = stat.tile([P, k], F32, tag="nrs")
            nc.gpsimd.tensor_scalar(nrs, rs, scalar1=-1.0, scalar2=None,
                                    op0=mybir.AluOpType.mult)
            mb = stat.tile([P, k], F32, tag="mb")
            nc.gpsimd.tensor_tensor(mb, mv[:, :, 0], nrs,
                                    op=mybir.AluOpType.mult)
            return mb

        def act_normalize(src, dst, mb, rs, gbc, bbc):
            nc.scalar.activation(
                dst, src, mybir.ActivationFunctionType.Identity,
                bias=mb, scale=rs)
            if gbc is not None:
                nc.gpsimd.tensor_mul(dst, dst, gbc)
                nc.gpsimd.tensor_add(dst, dst, bbc)

        def x0_batch(tiles):
            """LN-only tiles, pair-batched rsqrt; normalizes on GPSIMD
            (keeps the gelu-feeding scalar queue free of waits)."""
            k = len(tiles)
            mv = stat.tile([P, k, 2], F32, tag="mv0")
            for i, n in enumerate(tiles):
                ln_stats(x0sb[:, n], mv[:, i])
            rs = rsqrt_batch(mv[:, :, 1])
            for i, n in enumerate(tiles):
                o = wf32.tile([P, H], F32, tag="o")
                normalize(x0sb[:, n], o, mv[:, i], rs[:, i:i + 1],
                          nc.gpsimd, outgbc, outbbc)
                nc.sync.dma_start(d_y0[n * P:(n + 1) * P], o)

        # spread LN-only batches across mid-loop chunks (away from the ramp)
        x0_at = {}
        for b in range((nt0 + 1) // 2):
            tiles = list(range(2 * b, min(2 * b + 2, nt0)))
            x0_at.setdefault(min(1 + b, chunks - 1), []).append(tiles)

        def stage_b1(c, t1s, mv1, rs1):
            """norm1 -> +x residual -> LN2 stats/rsqrt for chunk c (one
            chunk behind stage A so every op's inputs are computed —
            avoids head-of-line stalls on the in-order queues)."""
            t2s = []
            mv2 = stat.tile([P, TPT, 2], F32, tag="mv2")
            for m in range(TPT):
                v1 = wbf.tile([P, H], BF16, tag="v1")
                normalize(t1s[m], v1, mv1[:, m], rs1[:, m:m + 1],
                          nc.vector, lngbc, lnbbc)
                t2 = wbf.tile([P, H], BF16, tag="t2")
                nc.gpsimd.tensor_tensor(t2, v1, xr0sb[:, c * TPT + m],
                                        op=mybir.AluOpType.add)
                ln_stats(t2, mv2[:, m])
                t2s.append(t2)
            rs2 = rsqrt_batch(mv2[:, :, 1])
            mb2 = neg_m_rs(mv2, rs2)
            return (c, t2s, mv2, rs2, mb2)

        def stage_b2(c, t2s, mv2, rs2, mb2):
            """final normalizes + stores for chunk c (two chunks behind:
            its deps are long since ready, so the scalar-queue normalize
            never blocks the gelus emitted after it)."""
            for m in range(TPT):
                o = wf32.tile([P, H], F32, tag="o")
                if m == 0:
                    act_normalize(t2s[m], o, mb2[:, 0:1], rs2[:, 0:1],
                                  outgbc, outbbc)
                else:
                    normalize(t2s[m], o, mv2[:, m], rs2[:, m:m + 1],
                              nc.gpsimd, outgbc, outbbc)
                n = c * TPT + m
                nc.sync.dma_start(d_yr[n * P:(n + 1) * P], o)

        pend_a = None   # awaiting stage_b1
        pend_b = None   # awaiting stage_b2
        for it in range(repeat * chunks):
            c = it % chunks
            if pend_b is not None:
                stage_b2(*pend_b)
                pend_b = None
            # --- GEMM1 + gelu: 6 batches of 4 kf-tiles ---
            has = []
            for q in range(NQ):
                ps1 = ps1p.tile([P, QK, TCH], F32, tag="ps1")
                for j in range(QK):
                    kf = q * QK + j
                    for t in range(KH // 2):
                        nc.tensor.matmul(
                            ps1[:, j],
                            w1sb[:, 2 * t:2 * t + 2, kf * P:(kf + 1) * P],
                            xrTsb[:, 2 * t:2 * t + 2, c * TCH:(c + 1) * TCH],
                            start=(t == 0), stop=(t == KH // 2 - 1),
                            perf_mode=mybir.MatmulPerfMode.DoubleRow)
                ha = hpool.tile([P, QK, TCH], FP8, tag="ha")
                if b1_zero:
                    nc.scalar.activation(ha, ps1, ACT_FUNC, bias=0.0, scale=ginv1)
                else:
                    for j in range(QK):
                        kf = q * QK + j
                        nc.scalar.activation(
                            ha[:, j], ps1[:, j], ACT_FUNC,
                            bias=b1sb[:, kf:kf + 1], scale=ginv1)
                has.append(ha)
            # --- GEMM2 per 128-token tile; LN stats pair-batched ---
            t1s = []
            mv1 = stat.tile([P, TPT, 2], F32, tag="mv1")
            for m in range(TPT):
                psU = psUp.tile([P, H], F32, tag="psU")
                for pr in range(KF // 2):
                    q, b = divmod(pr, QK // 2)
                    lhsT = has[q][:, 2 * b:2 * b + 2, m * P:(m + 1) * P]
                    for h0, h1 in ((0, 512), (512, H)):
                        nc.tensor.matmul(
                            psU[:, h0:h1], lhsT,
                            w2sb[:, 2 * pr:2 * pr + 2, h0:h1],
                            start=(pr == 0), stop=(pr == KF // 2 - 1),
                            perf_mode=mybir.MatmulPerfMode.DoubleRow)
                # t1 = sw2*(x+b2) + psU  (scale-invariant LN1 input);
                # releases psU as soon as it runs
                t1 = wbf.tile([P, H], BF16, tag="t1")
                nc.vector.scalar_tensor_tensor(
                    t1, xrsb[:, c * TPT + m], s2t, psU,
                    op0=mybir.AluOpType.mult, op1=mybir.AluOpType.add)
                ln_stats(t1, mv1[:, m])
                t1s.append(t1)
            rs1 = rsqrt_batch(mv1[:, :, 1])
            if pend_a is not None:
                pend_b = stage_b1(*pend_a)
            pend_a = (c, t1s, mv1, rs1)
            for tiles in x0_at.get(c, ()):
                x0_batch(tiles)
        if pend_b is not None:
            stage_b2(*pend_b)
        if pend_a is not None:
            stage_b2(*stage_b1(*pend_a))

    nc.finalize()
    return nc


_NC_CACHE: dict[tuple, bass.Bass] = {}


def get_nc(cap: int, cap0: int, repeat: int = 1,
           flags: tuple = (True, True, True, True)) -> bass.Bass:
    key = (cap, cap0, repeat, flags)
    if key not in _NC_CACHE:
        b1z, b2z, lnt, outt = flags
        _NC_CACHE[key] = build_nc(cap, cap0, repeat, b1_zero=b1z, b2_zero=b2z,
                                  ln_trivial=lnt, out_trivial=outt)
    return _NC_CACHE[key]


def _round_up(n: int, m: int) -> int:
    return max(m, ((n + m - 1) // m) * m)


def shard_inputs(input_tensor, type_seq, W1, b1, W2, b2, ln_g, ln_b, out_g, out_b):
    """Host-side routing + fp8/bf16 prep. Returns (in_maps, core_tokens,
    zero_splits, cap, cap0, flags)."""
    B, L, _H = input_tensor.shape
    assert _H == H, f"kernel hardcodes d_model={H}, got {_H}"
    x = np.ascontiguousarray(np.asarray(input_tensor, dtype=np.float32)).reshape(B * L, H)
    ts_flat = np.asarray(type_seq).reshape(-1).astype(np.int64)
    NB = W1.shape[0]
    per_expert = max(1, NCORES // NB)
    W1 = np.asarray(W1, dtype=np.float32)
    W2 = np.asarray(W2, dtype=np.float32)
    b1 = np.asarray(b1, dtype=np.float32)
    b2 = np.asarray(b2, dtype=np.float32)

    flags = (
        not b1.any(),
        not b2.any(),
        bool(np.all(ln_g == 1.0) and not np.asarray(ln_b).any()),
        bool(np.all(out_g == 1.0) and not np.asarray(out_b).any()),
    )

    core_tokens = []
    core_expert = []
    for e in range(NB):
        toks = np.nonzero(ts_flat == e + 1)[0]
        for s in np.array_split(toks, per_expert):
            core_tokens.append(s)
            core_expert.append(e)
    while len(core_tokens) < NCORES:
        core_tokens.append(np.zeros(0, dtype=np.int64))
        core_expert.append(0)
    zero_splits = np.array_split(np.nonzero(ts_flat == 0)[0], NCORES)

    cap = _round_up(max(len(t) for t in core_tokens), TCH)
    cap0 = _round_up(max(len(z) for z in zero_splits), P)

    sx = _pow2_scale(float(np.abs(x).max()))
    xq = (x * np.float32(sx)).astype(E4M3)  # global; sliced per core

    sw1 = [_pow2_scale(float(np.abs(W1[e]).max())) for e in range(NB)]
    sw2 = [_pow2_scale(float(np.abs(W2[e]).max())) for e in range(NB)]
    w1q = [(W1[e] * np.float32(sw1[e])).astype(E4M3) for e in range(NB)]
    w2q = [(W2[e] * np.float32(sw2[e])).astype(E4M3) for e in range(NB)]

    def f32c(a):
        return np.ascontiguousarray(np.asarray(a, dtype=np.float32))

    in_maps = []
    for cidx in range(NCORES):
        toks = core_tokens[cidx]
        e = core_expert[cidx]
        z = zero_splits[cidx]
        xrT = np.zeros((H, cap), E4M3)
        xrT[:, : len(toks)] = xq[toks].T
        xr = np.zeros((cap, H), NP_BF16)
        xr[: len(toks)] = (x[toks] + b2[e]).astype(NP_BF16)
        x0 = np.zeros((cap0, H), NP_BF16)
        x0[: len(z)] = x[z].astype(NP_BF16)
        im = {
            "xrT": np.ascontiguousarray(xrT),
            "w1": w1q[e],
            "w2": w2q[e],
            "xr": xr,
            "x0": x0,
            "ginv1": np.array([1.0 / (sx * sw1[e])], np.float32),
            "s2": np.array([sw2[e]], np.float32),
        }
        if not flags[0]:
            im["b1"] = f32c(b1[e])
        if not flags[1]:
            xr0 = np.zeros((cap, H), NP_BF16)
            xr0[: len(toks)] = x[toks].astype(NP_BF16)
            im["xr0"] = xr0
        if not flags[2]:
            im["lng"] = f32c(ln_g[e])
            im["lnb"] = f32c(ln_b[e])
        if not flags[3]:
            im["outg"] = f32c(out_g)
            im["outb"] = f32c(out_b)
        in_maps.append(im)
    return in_maps, core_tokens, zero_splits, cap, cap0, flags


def unshard_output(results, core_tokens, zero_splits, shape, dtype):
    B, L, _H = shape
    out = np.empty((B * L, H), np.float32)
    for c in range(NCORES):
        toks = core_tokens[c]
        z = zero_splits[c]
        if len(toks):
            out[toks] = results[c]["yr"][: len(toks)]
        if len(z):
            out[z] = results[c]["y0"][: len(z)]
    return out.reshape(B, L, H).astype(dtype, copy=False)


def kernel(input_tensor, type_seq, W1, b1, W2, b2, ln_g, ln_b, out_g, out_b):
    in_maps, core_tokens, zero_splits, cap, cap0, flags = shard_inputs(
        input_tensor, type_seq, W1, b1, W2, b2, ln_g, ln_b, out_g, out_b
    )
    nc = get_nc(cap, cap0, flags=flags)
    res = run_bass_kernel_spmd(nc, in_maps, core_ids=list(range(NCORES)))
    return unshard_output(
        res.results, core_tokens, zero_splits, input_tensor.shape,
        np.asarray(input_tensor).dtype,
    )


# revision 28
# speedup vs baseline: 1.0288x; 1.0288x over previous
"""Trainium2 Bass kernel for nn_MBSFeedForward (moe_routing) — fp8 edition.

Reference semantics (per token t with class c = type_seq[t]):
  c == 0:  out = LN_out(x_t)
  c >= 1:  e = c-1 (expert)
           u = GELU(x_t @ W1_e + b1_e) @ W2_e + b2_e
           v = LN_e(u + x_t)          (per-expert ln_g/ln_b)
           out = LN_out(v + x_t)      (out_g/out_b)

Sharding (host-side routing): 4 experts x 2 cores each; class-0 tokens
split over all 8 cores (outer LN only). No collectives; host scatters.

Device kernel design:
  - Both GEMMs run as fp8(e4m3) DoubleRow matmuls: each instruction
    contracts TWO 128-deep k-tiles (operands laid out [128, 2, n]) at
    0.5 PE cycles per output row. Host pre-quantizes x/W1/W2 with
    power-of-2 scales (RNE via ml_dtypes); the only on-device fp8
    rounding is the gelu->fp8 write, which the scalar engine does RNE
    (verified on HW). End-to-end rel err ~1.8e-2 vs the 2e-2 gate.
  - GEMM2 result arrives in PSUM scaled by sw2; LN is scale-invariant,
    so the first residual add computes sw2*(x+b2) + psU in one DVE
    scalar_tensor_tensor (sw2 rides along as a [P,1] scalar input) and
    the normalize needs no unscale. b2 folds into the residual array.
  - LayerNorm: bn_stats/bn_aggr on DVE, rsqrt via bit-trick + Newton on
    GPSIMD (keeps Sqrt off the scalar engine so the Gelu activation
    table is loaded exactly once), LN1 normalize on DVE (bf16 2x), the
    final normalize on GPSIMD writing f32.
  - All inputs are DMA'd to SBUF once, on the two HWDGE queues
    (SP/Activation) in the order chunk 0 consumes them; steady state
    does only output-store DMAs.
"""

import math
from contextlib import ExitStack

import numpy as np
import ml_dtypes

import concourse.bass as bass
import concourse.tile as tile
from concourse import bacc
from concourse import mybir
from concourse.bass_utils import run_bass_kernel_spmd

F32 = mybir.dt.float32
BF16 = mybir.dt.bfloat16
FP8 = mybir.dt.float8e4
I32 = mybir.dt.int32
E4M3 = ml_dtypes.float8_e4m3
NP_BF16 = ml_dtypes.bfloat16

P = 128
H = 768
F = 3072
KH = H // P     # 6  (k-tiles for GEMM1)
KF = F // P     # 24 (k-tiles for GEMM2)
TCH = 256       # tokens per chunk
TPT = TCH // P  # token tiles per chunk (2)
QK = 4          # kf-tiles per ps1/gelu batch
NQ = KF // QK   # 6 ps1 batches per chunk
NCORES = 8
RSQRT_MAGIC = 0x5F3759DF
FP8_TARGET = 192.0  # max|v*scale| target; e4m3 (IEEE) max finite is 240
NEWTON2 = True      # second Newton step for rsqrt

ACT_FUNC = mybir.ActivationFunctionType.Gelu


def _pow2_scale(m: float) -> float:
    if m <= 0 or not math.isfinite(m):
        return 1.0
    return 2.0 ** math.floor(math.log2(FP8_TARGET / m))


def build_nc(cap: int, cap0: int, repeat: int = 1, *, b1_zero: bool = True,
             b2_zero: bool = True, ln_trivial: bool = True,
             out_trivial: bool = True) -> bass.Bass:
    """Per-core Bass module for `cap` routed tokens and `cap0` LN-only
    tokens. repeat>1 re-runs the body (slope timing)."""
    assert cap % TCH == 0 and cap0 % P == 0
    nc = bacc.Bacc()
    chunks = cap // TCH
    nt0 = cap0 // P
    ntile = cap // P

    d_xrT = nc.dram_tensor("xrT", [H, cap], FP8, kind="ExternalInput")
    d_w1 = nc.dram_tensor("w1", [H, F], FP8, kind="ExternalInput")
    d_w2 = nc.dram_tensor("w2", [F, H], FP8, kind="ExternalInput")
    d_xr = nc.dram_tensor("xr", [cap, H], BF16, kind="ExternalInput")
    d_x0 = nc.dram_tensor("x0", [cap0, H], BF16, kind="ExternalInput")
    d_ginv1 = nc.dram_tensor("ginv1", [1], F32, kind="ExternalInput")
    d_s2 = nc.dram_tensor("s2", [1], F32, kind="ExternalInput")
    d_b1 = None if b1_zero else nc.dram_tensor("b1", [F], F32, kind="ExternalInput")
    # when b2 != 0, the residual-2 array (plain x) is separate from xr (x+b2)
    d_xr0 = None if b2_zero else nc.dram_tensor("xr0", [cap, H], BF16, kind="ExternalInput")
    d_lng = d_lnb = d_outg = d_outb = None
    if not ln_trivial:
        d_lng = nc.dram_tensor("lng", [H], F32, kind="ExternalInput")
        d_lnb = nc.dram_tensor("lnb", [H], F32, kind="ExternalInput")
    if not out_trivial:
        d_outg = nc.dram_tensor("outg", [H], F32, kind="ExternalInput")
        d_outb = nc.dram_tensor("outb", [H], F32, kind="ExternalInput")
    d_yr = nc.dram_tensor("yr", [cap, H], F32, kind="ExternalOutput")
    d_y0 = nc.dram_tensor("y0", [cap0, H], F32, kind="ExternalOutput")

    with tile.TileContext(nc) as tc, ExitStack() as ctx:
        singles = ctx.enter_context(tc.tile_pool(name="singles", bufs=1))
        hpool = ctx.enter_context(tc.tile_pool(name="hact", bufs=14))
        wbf = ctx.enter_context(tc.tile_pool(name="wbf", bufs=18))
        wf32 = ctx.enter_context(tc.tile_pool(name="wf32", bufs=8))
        stat = ctx.enter_context(tc.tile_pool(name="stat", bufs=48))
        ps1p = ctx.enter_context(tc.tile_pool(name="ps1", bufs=2, space="PSUM"))
        psUp = ctx.enter_context(tc.tile_pool(name="psU", bufs=2, space="PSUM"))

        # --- resident inputs on the two HWDGE queues, in the order chunk 0
        # consumes them: xrT c0, w1 (GEMM1), w2 (GEMM2), rest of xrT ---
        w1sb = singles.tile([P, KH, F], FP8)
        w1_v = d_w1[:].rearrange("(ko ki) f -> ki ko f", ki=P)
        w2sb = singles.tile([P, KF, H], FP8)
        w2_v = d_w2[:].rearrange("(ko ki) h -> ki ko h", ki=P)
        xrTsb = singles.tile([P, KH, cap], FP8)
        xrT_v = d_xrT[:].rearrange("(ko ki) t -> ki ko t", ki=P)
        c0 = min(3 * TCH, cap)
        nc.sync.dma_start(xrTsb[:, :, 0:c0], xrT_v[:, :, 0:c0])
        # interleave w1 thirds with w2 sixths so chunk 0's GEMM2 operands
        # land while its GEMM1 still runs
        for s in range(3):
            sl = slice(s * F // 3, (s + 1) * F // 3)
            nc.scalar.dma_start(w1sb[:, :, sl], w1_v[:, :, sl])
            for t in (2 * s, 2 * s + 1):
                tl = slice(t * KF // 6, (t + 1) * KF // 6)
                nc.scalar.dma_start(w2sb[:, tl], w2_v[:, tl])
        if c0 < cap:
            nc.sync.dma_start(xrTsb[:, :, c0:cap], xrT_v[:, :, c0:cap])

        x0sb = singles.tile([P, max(nt0, 1), H], BF16)
        if nt0:
            nc.sync.dma_start(x0sb[:, 0:nt0], d_x0[:].rearrange("(n p) h -> p n h", p=P))
        xrsb = singles.tile([P, ntile, H], BF16)
        xr_v = d_xr[:].rearrange("(n p) h -> p n h", p=P)
        nc.sync.dma_start(xrsb[:, 0:2], xr_v[:, 0:2])
        nc.sync.dma_start(xrsb[:, 2:ntile // 2], xr_v[:, 2:ntile // 2])
        nc.sync.dma_start(xrsb[:, ntile // 2:], xr_v[:, ntile // 2:])
        xr0sb = xrsb
        if not b2_zero:
            xr0sb = singles.tile([P, ntile, H], BF16, name="xr0sb")
            nc.sync.dma_start(xr0sb, d_xr0[:].rearrange("(n p) h -> p n h", p=P))

        ginv1 = singles.tile([P, 1], F32, name="ginv1")
        nc.gpsimd.dma_start(ginv1, d_ginv1[:][None, :].to_broadcast([P, 1]))
        s2t = singles.tile([P, 1], F32, name="s2t")
        nc.gpsimd.dma_start(s2t, d_s2[:][None, :].to_broadcast([P, 1]))

        def bc_tile(d, nm):
            t = singles.tile([P, H], F32, tag=nm, name=nm)
            nc.gpsimd.dma_start(t, d[:][None, :].to_broadcast([P, H]))
            return t

        lngbc = lnbbc = outgbc = outbbc = None
        if not ln_trivial:
            lngbc, lnbbc = bc_tile(d_lng, "lngbc"), bc_tile(d_lnb, "lnbbc")
        if not out_trivial:
            outgbc, outbbc = bc_tile(d_outg, "outgbc"), bc_tile(d_outb, "outbbc")
        b1sb = None
        if not b1_zero:
            b1sb = singles.tile([P, KF], F32, name="b1sb")
            nc.gpsimd.dma_start(b1sb, d_b1[:].rearrange("(o p) -> p o", p=P))

        def ln_stats(src, mv):
            """DVE: bn_stats+aggr of src [P, H] into mv ([P, 2] slice)."""
            st = stat.tile([P, 2, 6], F32, tag="st")
            nc.vector.bn_stats(st[:, 0], src[:, 0:384])
            nc.vector.bn_stats(st[:, 1], src[:, 384:768])
            nc.vector.bn_aggr(mv, st)

        def rsqrt_batch(v):
            """rs = 1/sqrt(v) elementwise for v [P, k] (k small). Quake seed
            y0 = float_bits(magic - (v_int >> 1)) built as (~(v>>1)) +
            (magic+1) — shift+xor fuse (bitwise pair) and there is no
            reversed subtract; GPSIMD fails the ISA check for shifts so the
            seed runs on DVE. Newton steps y' = (1.5 - 0.5*v*y^2)*y go on
            GPSIMD as stt/tt/stt with signs folded into the constants."""
            k = v.shape[-1]
            yi = stat.tile([P, k], I32, tag="yi")
            nc.vector.tensor_scalar(
                yi, v.bitcast(I32), scalar1=1, scalar2=-1,
                op0=mybir.AluOpType.logical_shift_right,
                op1=mybir.AluOpType.bitwise_xor)
            nc.vector.tensor_scalar(
                yi, yi, scalar1=RSQRT_MAGIC + 1, scalar2=None,
                op0=mybir.AluOpType.add)
            rs = yi.bitcast(F32)
            for _ in range(1 + int(NEWTON2)):
                z = stat.tile([P, k], F32, tag="z")
                nc.gpsimd.tensor_tensor(z, rs, rs, op=mybir.AluOpType.mult)
                nc.gpsimd.tensor_tensor(z, z, v, op=mybir.AluOpType.mult)
                nc.gpsimd.tensor_scalar(
                    z, z, scalar1=-0.5, scalar2=-1.5,
                    op0=mybir.AluOpType.mult, op1=mybir.AluOpType.subtract)
                rs_n = stat.tile([P, k], F32, tag="rsn")
                nc.gpsimd.tensor_tensor(rs_n, z, rs, op=mybir.AluOpType.mult)
                rs = rs_n
            return rs

        def normalize(src, dst, mv, rs, eng, gbc, bbc):
            eng.tensor_scalar(
                dst, src, scalar1=mv[:, 0:1], scalar2=rs,
                op0=mybir.AluOpType.subtract, op1=mybir.AluOpType.mult)
            if gbc is not None:
                nc.gpsimd.tensor_mul(dst, dst, gbc)
                nc.gpsimd.tensor_add(dst, dst, bbc)

        def neg_m_rs(mv, rs):
            """[P,k] bias tile -mean*rs for activation-engine normalizes:
            activation(Identity, scale=rs, bias=-m*rs) == (t - m) * rs."""
            k = rs.shape[-1]
            nrs = stat.tile([P, k], F32, tag="nrs")
            nc.gpsimd.tensor_scalar(nrs, rs, scalar1=-1.0, scalar2=None,
                                    op0=mybir.AluOpType.mult)
            mb = stat.tile([P, k], F32, tag="mb")
            nc.gpsimd.tensor_tensor(mb, mv[:, :, 0], nrs,
                                    op=mybir.AluOpType.mult)
            return mb

        def act_normalize(src, dst, mb, rs, gbc, bbc):
            nc.scalar.activation(
                dst, src, mybir.ActivationFunctionType.Identity,
                bias=mb, scale=rs)
            if gbc is not None:
                nc.gpsimd.tensor_mul(dst, dst, gbc)
                nc.gpsimd.tensor_add(dst, dst, bbc)

        def x0_batch(tiles):
            """LN-only tiles, pair-batched rsqrt; normalizes on GPSIMD
            (keeps the gelu-feeding scalar queue free of waits)."""
            k = len(tiles)
            mv = stat.tile([P, k, 2], F32, tag="mv0")
            for i, n in enumerate(tiles):
                ln_stats(x0sb[:, n], mv[:, i])
            rs = rsqrt_batch(mv[:, :, 1])
            for i, n in enumerate(tiles):
                o = wf32.tile([P, H], F32, tag="o")
                normalize(x0sb[:, n], o, mv[:, i], rs[:, i:i + 1],
                          nc.gpsimd, outgbc, outbbc)
                nc.sync.dma_start(d_y0[n * P:(n + 1) * P], o)

        # spread LN-only batches across mid-loop chunks (away from the ramp)
        x0_at = {}
        for b in range((nt0 + 1) // 2):
            tiles = list(range(2 * b, min(2 * b + 2, nt0)))
            x0_at.setdefault(min(b, chunks - 1), []).append(tiles)

        def stage_b1(c, t1s, mv1, rs1):
            """norm1 -> +x residual -> LN2 stats/rsqrt for chunk c (one
            chunk behind stage A so every op's inputs are computed —
            avoids head-of-line stalls on the in-order queues)."""
            t2s = []
            mv2 = stat.tile([P, TPT, 2], F32, tag="mv2")
            for m in range(TPT):
                v1 = wbf.tile([P, H], BF16, tag="v1")
                normalize(t1s[m], v1, mv1[:, m], rs1[:, m:m + 1],
                          nc.vector, lngbc, lnbbc)
                t2 = wbf.tile([P, H], BF16, tag="t2")
                nc.gpsimd.tensor_tensor(t2, v1, xr0sb[:, c * TPT + m],
                                        op=mybir.AluOpType.add)
                ln_stats(t2, mv2[:, m])
                t2s.append(t2)
            rs2 = rsqrt_batch(mv2[:, :, 1])
            mb2 = neg_m_rs(mv2, rs2)
            return (c, t2s, mv2, rs2, mb2)

        def stage_b2(c, t2s, mv2, rs2, mb2):
            """final normalizes + stores for chunk c (two chunks behind:
            its deps are long since ready, so the scalar-queue normalize
            never blocks the gelus emitted after it)."""
            for m in range(TPT):
                o = wf32.tile([P, H], F32, tag="o")
                normalize(t2s[m], o, mv2[:, m], rs2[:, m:m + 1],
                          nc.gpsimd, outgbc, outbbc)
                n = c * TPT + m
                nc.sync.dma_start(d_yr[n * P:(n + 1) * P], o)

        pend_a = None   # awaiting stage_b1+b2
        for it in range(repeat * chunks):
            c = it % chunks
            # --- GEMM1 + gelu: 6 batches of 4 kf-tiles ---
            has = []
            for q in range(NQ):
                ps1 = ps1p.tile([P, QK, TCH], F32, tag="ps1")
                for j in range(QK):
                    kf = q * QK + j
                    for t in range(KH // 2):
                        nc.tensor.matmul(
                            ps1[:, j],
                            w1sb[:, 2 * t:2 * t + 2, kf * P:(kf + 1) * P],
                            xrTsb[:, 2 * t:2 * t + 2, c * TCH:(c + 1) * TCH],
                            start=(t == 0), stop=(t == KH // 2 - 1),
                            perf_mode=mybir.MatmulPerfMode.DoubleRow)
                ha = hpool.tile([P, QK, TCH], FP8, tag="ha")
                if b1_zero:
                    nc.scalar.activation(ha, ps1, ACT_FUNC, bias=0.0, scale=ginv1)
                else:
                    for j in range(QK):
                        kf = q * QK + j
                        nc.scalar.activation(
                            ha[:, j], ps1[:, j], ACT_FUNC,
                            bias=b1sb[:, kf:kf + 1], scale=ginv1)
                has.append(ha)
            # --- GEMM2 per 128-token tile; LN stats pair-batched ---
            t1s = []
            mv1 = stat.tile([P, TPT, 2], F32, tag="mv1")
            for m in range(TPT):
                psU = psUp.tile([P, H], F32, tag="psU")
                for pr in range(KF // 2):
                    q, b = divmod(pr, QK // 2)
                    lhsT = has[q][:, 2 * b:2 * b + 2, m * P:(m + 1) * P]
                    for h0, h1 in ((0, 512), (512, H)):
                        nc.tensor.matmul(
                            psU[:, h0:h1], lhsT,
                            w2sb[:, 2 * pr:2 * pr + 2, h0:h1],
                            start=(pr == 0), stop=(pr == KF // 2 - 1),
                            perf_mode=mybir.MatmulPerfMode.DoubleRow)
                # t1 = sw2*(x+b2) + psU  (scale-invariant LN1 input);
                # releases psU as soon as it runs
                t1 = wbf.tile([P, H], BF16, tag="t1")
                nc.vector.scalar_tensor_tensor(
                    t1, xrsb[:, c * TPT + m], s2t, psU,
                    op0=mybir.AluOpType.mult, op1=mybir.AluOpType.add)
                ln_stats(t1, mv1[:, m])
                t1s.append(t1)
            rs1 = rsqrt_batch(mv1[:, :, 1])
            if pend_a is not None:
                stage_b2(*stage_b1(*pend_a))
            pend_a = (c, t1s, mv1, rs1)
            for tiles in x0_at.get(c, ()):
                x0_batch(tiles)
        if pend_a is not None:
            stage_b2(*stage_b1(*pend_a))

    nc.finalize()
    return nc


_NC_CACHE: dict[tuple, bass.Bass] = {}


def get_nc(cap: int, cap0: int, repeat: int = 1,
           flags: tuple = (True, True, True, True)) -> bass.Bass:
    key = (cap, cap0, repeat, flags)
    if key not in _NC_CACHE:
        b1z, b2z, lnt, outt = flags
        _NC_CACHE[key] = build_nc(cap, cap0, repeat, b1_zero=b1z, b2_zero=b2z,
                                  ln_trivial=lnt, out_trivial=outt)
    return _NC_CACHE[key]


def _round_up(n: int, m: int) -> int:
    return max(m, ((n + m - 1) // m) * m)


def shard_inputs(input_tensor, type_seq, W1, b1, W2, b2, ln_g, ln_b, out_g, out_b):
    """Host-side routing + fp8/bf16 prep. Returns (in_maps, core_tokens,
    zero_splits, cap, cap0, flags)."""
    B, L, _H = input_tensor.shape
    assert _H == H, f"kernel hardcodes d_model={H}, got {_H}"
    x = np.ascontiguousarray(np.asarray(input_tensor, dtype=np.float32)).reshape(B * L, H)
    ts_flat = np.asarray(type_seq).reshape(-1).astype(np.int64)
    NB = W1.shape[0]
    per_expert = max(1, NCORES // NB)
    W1 = np.asarray(W1, dtype=np.float32)
    W2 = np.asarray(W2, dtype=np.float32)
    b1 = np.asarray(b1, dtype=np.float32)
    b2 = np.asarray(b2, dtype=np.float32)

    flags = (
        not b1.any(),
        not b2.any(),
        bool(np.all(ln_g == 1.0) and not np.asarray(ln_b).any()),
        bool(np.all(out_g == 1.0) and not np.asarray(out_b).any()),
    )

    core_tokens = []
    core_expert = []
    for e in range(NB):
        toks = np.nonzero(ts_flat == e + 1)[0]
        for s in np.array_split(toks, per_expert):
            core_tokens.append(s)
            core_expert.append(e)
    while len(core_tokens) < NCORES:
        core_tokens.append(np.zeros(0, dtype=np.int64))
        core_expert.append(0)
    zero_splits = np.array_split(np.nonzero(ts_flat == 0)[0], NCORES)

    cap = _round_up(max(len(t) for t in core_tokens), TCH)
    cap0 = _round_up(max(len(z) for z in zero_splits), P)

    sx = _pow2_scale(float(np.abs(x).max()))
    xq = (x * np.float32(sx)).astype(E4M3)  # global; sliced per core

    sw1 = [_pow2_scale(float(np.abs(W1[e]).max())) for e in range(NB)]
    sw2 = [_pow2_scale(float(np.abs(W2[e]).max())) for e in range(NB)]
    w1q = [(W1[e] * np.float32(sw1[e])).astype(E4M3) for e in range(NB)]
    w2q = [(W2[e] * np.float32(sw2[e])).astype(E4M3) for e in range(NB)]

    def f32c(a):
        return np.ascontiguousarray(np.asarray(a, dtype=np.float32))

    in_maps = []
    for cidx in range(NCORES):
        toks = core_tokens[cidx]
        e = core_expert[cidx]
        z = zero_splits[cidx]
        xrT = np.zeros((H, cap), E4M3)
        xrT[:, : len(toks)] = xq[toks].T
        xr = np.zeros((cap, H), NP_BF16)
        xr[: len(toks)] = (x[toks] + b2[e]).astype(NP_BF16)
        x0 = np.zeros((cap0, H), NP_BF16)
        x0[: len(z)] = x[z].astype(NP_BF16)
        im = {
            "xrT": np.ascontiguousarray(xrT),
            "w1": w1q[e],
            "w2": w2q[e],
            "xr": xr,
            "x0": x0,
            "ginv1": np.array([1.0 / (sx * sw1[e])], np.float32),
            "s2": np.array([sw2[e]], np.float32),
        }
        if not flags[0]:
            im["b1"] = f32c(b1[e])
        if not flags[1]:
            xr0 = np.zeros((cap, H), NP_BF16)
            xr0[: len(toks)] = x[toks].astype(NP_BF16)
            im["xr0"] = xr0
        if not flags[2]:
            im["lng"] = f32c(ln_g[e])
            im["lnb"] = f32c(ln_b[e])
        if not flags[3]:
            im["outg"] = f32c(out_g)
            im["outb"] = f32c(out_b)
        in_maps.append(im)
    return in_maps, core_tokens, zero_splits, cap, cap0, flags


def unshard_output(results, core_tokens, zero_splits, shape, dtype):
    B, L, _H = shape
    out = np.empty((B * L, H), np.float32)
    for c in range(NCORES):
        toks = core_tokens[c]
        z = zero_splits[c]
        if len(toks):
            out[toks] = results[c]["yr"][: len(toks)]
        if len(z):
            out[z] = results[c]["y0"][: len(z)]
    return out.reshape(B, L, H).astype(dtype, copy=False)


def kernel(input_tensor, type_seq, W1, b1, W2, b2, ln_g, ln_b, out_g, out_b):
    in_maps, core_tokens, zero_splits, cap, cap0, flags = shard_inputs(
        input_tensor, type_seq, W1, b1, W2, b2, ln_g, ln_b, out_g, out_b
    )
    nc = get_nc(cap, cap0, flags=flags)
    res = run_bass_kernel_spmd(nc, in_maps, core_ids=list(range(NCORES)))
    return unshard_output(
        res.results, core_tokens, zero_splits, input_tensor.shape,
        np.asarray(input_tensor).dtype,
    )


# revision 30
# speedup vs baseline: 1.0569x; 1.0273x over previous
"""Trainium2 Bass kernel for nn_MBSFeedForward (moe_routing) — fp8 edition.

Reference semantics (per token t with class c = type_seq[t]):
  c == 0:  out = LN_out(x_t)
  c >= 1:  e = c-1 (expert)
           u = GELU(x_t @ W1_e + b1_e) @ W2_e + b2_e
           v = LN_e(u + x_t)          (per-expert ln_g/ln_b)
           out = LN_out(v + x_t)      (out_g/out_b)

Sharding (host-side routing): 4 experts x 2 cores each; class-0 tokens
split over all 8 cores (outer LN only). No collectives; host scatters.

Device kernel design:
  - Both GEMMs run as fp8(e4m3) DoubleRow matmuls: each instruction
    contracts TWO 128-deep k-tiles (operands laid out [128, 2, n]) at
    0.5 PE cycles per output row. Host pre-quantizes x/W1/W2 with
    power-of-2 scales (RNE via ml_dtypes); the only on-device fp8
    rounding is the gelu->fp8 write, which the scalar engine does RNE
    (verified on HW). End-to-end rel err ~1.8e-2 vs the 2e-2 gate.
  - GEMM2 result arrives in PSUM scaled by sw2; LN is scale-invariant,
    so the first residual add computes sw2*(x+b2) + psU in one DVE
    scalar_tensor_tensor (sw2 rides along as a [P,1] scalar input) and
    the normalize needs no unscale. b2 folds into the residual array.
  - LayerNorm: bn_stats/bn_aggr on DVE, rsqrt via bit-trick + Newton on
    GPSIMD (keeps Sqrt off the scalar engine so the Gelu activation
    table is loaded exactly once), LN1 normalize on DVE (bf16 2x), the
    final normalize on GPSIMD writing f32.
  - All inputs are DMA'd to SBUF once, on the two HWDGE queues
    (SP/Activation) in the order chunk 0 consumes them; steady state
    does only output-store DMAs.
"""

import math
from contextlib import ExitStack

import numpy as np
import ml_dtypes

import concourse.bass as bass
import concourse.tile as tile
from concourse import bacc
from concourse import mybir
from concourse.bass_utils import run_bass_kernel_spmd

F32 = mybir.dt.float32
BF16 = mybir.dt.bfloat16
FP8 = mybir.dt.float8e4
I32 = mybir.dt.int32
E4M3 = ml_dtypes.float8_e4m3
NP_BF16 = ml_dtypes.bfloat16

P = 128
H = 768
F = 3072
KH = H // P     # 6  (k-tiles for GEMM1)
KF = F // P     # 24 (k-tiles for GEMM2)
TCH = 256       # tokens per chunk
TPT = TCH // P  # token tiles per chunk (2)
QK = 4          # kf-tiles per ps1/gelu batch
NQ = KF // QK   # 6 ps1 batches per chunk
NCORES = 8
RSQRT_MAGIC = 0x5F3759DF
FP8_TARGET = 192.0  # max|v*scale| target; e4m3 (IEEE) max finite is 240
NEWTON2 = True      # second Newton step for rsqrt

ACT_FUNC = mybir.ActivationFunctionType.Gelu


def _pow2_scale(m: float) -> float:
    if m <= 0 or not math.isfinite(m):
        return 1.0
    return 2.0 ** math.floor(math.log2(FP8_TARGET / m))


def build_nc(cap: int, cap0: int, repeat: int = 1, *, b1_zero: bool = True,
             b2_zero: bool = True, ln_trivial: bool = True,
             out_trivial: bool = True) -> bass.Bass:
    """Per-core Bass module for `cap` routed tokens and `cap0` LN-only
    tokens. repeat>1 re-runs the body (slope timing)."""
    assert cap % TCH == 0 and cap0 % P == 0
    nc = bacc.Bacc()
    chunks = cap // TCH
    nt0 = cap0 // P
    ntile = cap // P

    d_xrT = nc.dram_tensor("xrT", [H, cap], FP8, kind="ExternalInput")
    d_w1 = nc.dram_tensor("w1", [H, F], FP8, kind="ExternalInput")
    d_w2 = nc.dram_tensor("w2", [F, H], FP8, kind="ExternalInput")
    d_xr = nc.dram_tensor("xr", [cap, H], BF16, kind="ExternalInput")
    d_x0 = nc.dram_tensor("x0", [cap0, H], BF16, kind="ExternalInput")
    d_ginv1 = nc.dram_tensor("ginv1", [1], F32, kind="ExternalInput")
    d_s2 = nc.dram_tensor("s2", [1], F32, kind="ExternalInput")
    d_b1 = None if b1_zero else nc.dram_tensor("b1", [F], F32, kind="ExternalInput")
    # when b2 != 0, the residual-2 array (plain x) is separate from xr (x+b2)
    d_xr0 = None if b2_zero else nc.dram_tensor("xr0", [cap, H], BF16, kind="ExternalInput")
    d_lng = d_lnb = d_outg = d_outb = None
    if not ln_trivial:
        d_lng = nc.dram_tensor("lng", [H], F32, kind="ExternalInput")
        d_lnb = nc.dram_tensor("lnb", [H], F32, kind="ExternalInput")
    if not out_trivial:
        d_outg = nc.dram_tensor("outg", [H], F32, kind="ExternalInput")
        d_outb = nc.dram_tensor("outb", [H], F32, kind="ExternalInput")
    d_yr = nc.dram_tensor("yr", [cap, H], F32, kind="ExternalOutput")
    d_y0 = nc.dram_tensor("y0", [cap0, H], F32, kind="ExternalOutput")

    with tile.TileContext(nc) as tc, ExitStack() as ctx:
        singles = ctx.enter_context(tc.tile_pool(name="singles", bufs=1))
        hpool = ctx.enter_context(tc.tile_pool(name="hact", bufs=14))
        wbf = ctx.enter_context(tc.tile_pool(name="wbf", bufs=12))
        wf32 = ctx.enter_context(tc.tile_pool(name="wf32", bufs=8))
        stat = ctx.enter_context(tc.tile_pool(name="stat", bufs=48))
        ps1p = ctx.enter_context(tc.tile_pool(name="ps1", bufs=2, space="PSUM"))
        psUp = ctx.enter_context(tc.tile_pool(name="psU", bufs=2, space="PSUM"))

        # --- resident inputs on the two HWDGE queues, in the order chunk 0
        # consumes them: xrT c0, w1 (GEMM1), w2 (GEMM2), rest of xrT ---
        w1sb = singles.tile([P, KH, F], FP8)
        w1_v = d_w1[:].rearrange("(ko ki) f -> ki ko f", ki=P)
        w2sb = singles.tile([P, KF, H], FP8)
        w2_v = d_w2[:].rearrange("(ko ki) h -> ki ko h", ki=P)
        xrTsb = singles.tile([P, KH, cap], FP8)
        xrT_v = d_xrT[:].rearrange("(ko ki) t -> ki ko t", ki=P)
        c0 = min(3 * TCH, cap)
        nc.sync.dma_start(xrTsb[:, :, 0:c0], xrT_v[:, :, 0:c0])
        # interleave w1 thirds with w2 sixths so chunk 0's GEMM2 operands
        # land while its GEMM1 still runs
        for s in range(3):
            sl = slice(s * F // 3, (s + 1) * F // 3)
            nc.scalar.dma_start(w1sb[:, :, sl], w1_v[:, :, sl])
            for t in (2 * s, 2 * s + 1):
                tl = slice(t * KF // 6, (t + 1) * KF // 6)
                nc.scalar.dma_start(w2sb[:, tl], w2_v[:, tl])
        if c0 < cap:
            nc.sync.dma_start(xrTsb[:, :, c0:cap], xrT_v[:, :, c0:cap])

        x0sb = singles.tile([P, max(nt0, 1), H], BF16)
        if nt0:
            nc.sync.dma_start(x0sb[:, 0:nt0], d_x0[:].rearrange("(n p) h -> p n h", p=P))
        xrsb = singles.tile([P, ntile, H], BF16)
        xr_v = d_xr[:].rearrange("(n p) h -> p n h", p=P)
        nc.sync.dma_start(xrsb[:, 0:2], xr_v[:, 0:2])
        nc.sync.dma_start(xrsb[:, 2:ntile // 2], xr_v[:, 2:ntile // 2])
        nc.sync.dma_start(xrsb[:, ntile // 2:], xr_v[:, ntile // 2:])
        xr0sb = xrsb
        if not b2_zero:
            xr0sb = singles.tile([P, ntile, H], BF16, name="xr0sb")
            nc.sync.dma_start(xr0sb, d_xr0[:].rearrange("(n p) h -> p n h", p=P))

        ginv1 = singles.tile([P, 1], F32, name="ginv1")
        nc.gpsimd.dma_start(ginv1, d_ginv1[:][None, :].to_broadcast([P, 1]))
        s2t = singles.tile([P, 1], F32, name="s2t")
        nc.gpsimd.dma_start(s2t, d_s2[:][None, :].to_broadcast([P, 1]))

        def bc_tile(d, nm):
            t = singles.tile([P, H], F32, tag=nm, name=nm)
            nc.gpsimd.dma_start(t, d[:][None, :].to_broadcast([P, H]))
            return t

        lngbc = lnbbc = outgbc = outbbc = None
        if not ln_trivial:
            lngbc, lnbbc = bc_tile(d_lng, "lngbc"), bc_tile(d_lnb, "lnbbc")
        if not out_trivial:
            outgbc, outbbc = bc_tile(d_outg, "outgbc"), bc_tile(d_outb, "outbbc")
        b1sb = None
        if not b1_zero:
            b1sb = singles.tile([P, KF], F32, name="b1sb")
            nc.gpsimd.dma_start(b1sb, d_b1[:].rearrange("(o p) -> p o", p=P))

        def sumsq(src, acc):
            """DVE: acc[P,1] = sum(src^2)/H via tensor_tensor_reduce (bf16
            operands keep the 2x DVE mode; the product tensor goes to a
            write-only scratch)."""
            sq = wbf.tile([P, H], BF16, tag="sq")
            nc.vector.tensor_tensor_reduce(
                sq, src, src, 1.0 / H, 0.0,
                op0=mybir.AluOpType.mult, op1=mybir.AluOpType.add,
                accum_out=acc)

        def var_chain(tsum, s2s):
            """Pool: mean = tsum/H, var = s2s - mean^2 for [P,k] batches.
            Means here are << std (residual streams are near zero-mean per
            token), so the subtraction loses no precision."""
            k = tsum.shape[-1]
            mean = stat.tile([P, k], F32, tag="mean")
            nc.gpsimd.tensor_scalar(mean, tsum, scalar1=1.0 / H, scalar2=None,
                                    op0=mybir.AluOpType.mult)
            z = stat.tile([P, k], F32, tag="zm")
            nc.gpsimd.tensor_tensor(z, mean, mean, op=mybir.AluOpType.mult)
            vv = stat.tile([P, k], F32, tag="vv")
            nc.gpsimd.tensor_tensor(vv, s2s, z, op=mybir.AluOpType.subtract)
            return mean, vv

        def rsqrt_batch(v):
            """rs = 1/sqrt(v) elementwise for v [P, k] (k small). Quake seed
            y0 = float_bits(magic - (v_int >> 1)) built as (~(v>>1)) +
            (magic+1) — shift+xor fuse (bitwise pair) and there is no
            reversed subtract; GPSIMD fails the ISA check for shifts so the
            seed runs on DVE. Newton steps y' = (1.5 - 0.5*v*y^2)*y go on
            GPSIMD as stt/tt/stt with signs folded into the constants."""
            k = v.shape[-1]
            yi = stat.tile([P, k], I32, tag="yi")
            nc.vector.tensor_scalar(
                yi, v.bitcast(I32), scalar1=1, scalar2=-1,
                op0=mybir.AluOpType.logical_shift_right,
                op1=mybir.AluOpType.bitwise_xor)
            nc.vector.tensor_scalar(
                yi, yi, scalar1=RSQRT_MAGIC + 1, scalar2=None,
                op0=mybir.AluOpType.add)
            rs = yi.bitcast(F32)
            for _ in range(1 + int(NEWTON2)):
                z = stat.tile([P, k], F32, tag="z")
                nc.gpsimd.tensor_tensor(z, rs, rs, op=mybir.AluOpType.mult)
                nc.gpsimd.tensor_tensor(z, z, v, op=mybir.AluOpType.mult)
                nc.gpsimd.tensor_scalar(
                    z, z, scalar1=-0.5, scalar2=-1.5,
                    op0=mybir.AluOpType.mult, op1=mybir.AluOpType.subtract)
                rs_n = stat.tile([P, k], F32, tag="rsn")
                nc.gpsimd.tensor_tensor(rs_n, z, rs, op=mybir.AluOpType.mult)
                rs = rs_n
            return rs

        def normalize(src, dst, mean, rs, eng, gbc, bbc):
            eng.tensor_scalar(
                dst, src, scalar1=mean, scalar2=rs,
                op0=mybir.AluOpType.subtract, op1=mybir.AluOpType.mult)
            if gbc is not None:
                nc.gpsimd.tensor_mul(dst, dst, gbc)
                nc.gpsimd.tensor_add(dst, dst, bbc)

        def neg_m_rs(mv, rs):
            """[P,k] bias tile -mean*rs for activation-engine normalizes:
            activation(Identity, scale=rs, bias=-m*rs) == (t - m) * rs."""
            k = rs.shape[-1]
            nrs = stat.tile([P, k], F32, tag="nrs")
            nc.gpsimd.tensor_scalar(nrs, rs, scalar1=-1.0, scalar2=None,
                                    op0=mybir.AluOpType.mult)
            mb = stat.tile([P, k], F32, tag="mb")
            nc.gpsimd.tensor_tensor(mb, mv[:, :, 0], nrs,
                                    op=mybir.AluOpType.mult)
            return mb

        def act_normalize(src, dst, mb, rs, gbc, bbc):
            nc.scalar.activation(
                dst, src, mybir.ActivationFunctionType.Identity,
                bias=mb, scale=rs)
            if gbc is not None:
                nc.gpsimd.tensor_mul(dst, dst, gbc)
                nc.gpsimd.tensor_add(dst, dst, bbc)

        def x0_batch(tiles):
            """LN-only tiles, pair-batched rsqrt; normalizes on GPSIMD
            (keeps the gelu-feeding scalar queue free of waits)."""
            k = len(tiles)
            tsum = stat.tile([P, k], F32, tag="tsum0")
            s2s = stat.tile([P, k], F32, tag="s2s0")
            for i, n in enumerate(tiles):
                nc.vector.tensor_reduce(
                    tsum[:, i:i + 1], x0sb[:, n], mybir.AxisListType.X,
                    mybir.AluOpType.add)
                sumsq(x0sb[:, n], s2s[:, i:i + 1])
            mean, vv = var_chain(tsum, s2s)
            rs = rsqrt_batch(vv)
            for i, n in enumerate(tiles):
                o = wf32.tile([P, H], F32, tag="o")
                normalize(x0sb[:, n], o, mean[:, i:i + 1], rs[:, i:i + 1],
                          nc.gpsimd, outgbc, outbbc)
                nc.sync.dma_start(d_y0[n * P:(n + 1) * P], o)

        # spread LN-only batches across mid-loop chunks (away from the ramp)
        x0_at = {}
        for b in range((nt0 + 1) // 2):
            tiles = list(range(2 * b, min(2 * b + 2, nt0)))
            x0_at.setdefault(min(b, chunks - 1), []).append(tiles)

        def stage_b1(c, t1s, mean1, rs1):
            """norm1 -> +x residual -> LN2 stats/rsqrt for chunk c (one
            chunk behind stage A so every op's inputs are computed —
            avoids head-of-line stalls on the in-order queues)."""
            t2s = []
            tsum2 = stat.tile([P, TPT], F32, tag="tsum2")
            s2s2 = stat.tile([P, TPT], F32, tag="s2s2")
            for m in range(TPT):
                v1 = wbf.tile([P, H], BF16, tag="v1")
                normalize(t1s[m], v1, mean1[:, m:m + 1], rs1[:, m:m + 1],
                          nc.vector if m == 0 else nc.gpsimd, lngbc, lnbbc)
                t2 = wbf.tile([P, H], BF16, tag="t2")
                nc.vector.scalar_tensor_tensor(
                    t2, v1, 1.0, xr0sb[:, c * TPT + m],
                    op0=mybir.AluOpType.mult, op1=mybir.AluOpType.add,
                    accum_out=tsum2[:, m:m + 1])
                sumsq(t2, s2s2[:, m:m + 1])
                t2s.append(t2)
            mean2, vv2 = var_chain(tsum2, s2s2)
            rs2 = rsqrt_batch(vv2)
            return (c, t2s, mean2, rs2)

        def stage_b2(c, t2s, mean2, rs2):
            """final normalizes + stores for chunk c."""
            for m in range(TPT):
                o = wf32.tile([P, H], F32, tag="o")
                normalize(t2s[m], o, mean2[:, m:m + 1], rs2[:, m:m + 1],
                          nc.gpsimd, outgbc, outbbc)
                n = c * TPT + m
                nc.sync.dma_start(d_yr[n * P:(n + 1) * P], o)

        pend_a = None   # awaiting stage_b1+b2
        for it in range(repeat * chunks):
            c = it % chunks
            # --- GEMM1 + gelu: 6 batches of 4 kf-tiles ---
            has = []
            for q in range(NQ):
                ps1 = ps1p.tile([P, QK, TCH], F32, tag="ps1")
                for j in range(QK):
                    kf = q * QK + j
                    for t in range(KH // 2):
                        nc.tensor.matmul(
                            ps1[:, j],
                            w1sb[:, 2 * t:2 * t + 2, kf * P:(kf + 1) * P],
                            xrTsb[:, 2 * t:2 * t + 2, c * TCH:(c + 1) * TCH],
                            start=(t == 0), stop=(t == KH // 2 - 1),
                            perf_mode=mybir.MatmulPerfMode.DoubleRow)
                ha = hpool.tile([P, QK, TCH], FP8, tag="ha")
                if b1_zero:
                    nc.scalar.activation(ha, ps1, ACT_FUNC, bias=0.0, scale=ginv1)
                else:
                    for j in range(QK):
                        kf = q * QK + j
                        nc.scalar.activation(
                            ha[:, j], ps1[:, j], ACT_FUNC,
                            bias=b1sb[:, kf:kf + 1], scale=ginv1)
                has.append(ha)
            # --- GEMM2 per 128-token tile; LN stats pair-batched ---
            t1s = []
            tsum1 = stat.tile([P, TPT], F32, tag="tsum1")
            s2s1 = stat.tile([P, TPT], F32, tag="s2s1")
            for m in range(TPT):
                psU = psUp.tile([P, H], F32, tag="psU")
                for pr in range(KF // 2):
                    q, b = divmod(pr, QK // 2)
                    lhsT = has[q][:, 2 * b:2 * b + 2, m * P:(m + 1) * P]
                    for h0, h1 in ((0, 512), (512, H)):
                        nc.tensor.matmul(
                            psU[:, h0:h1], lhsT,
                            w2sb[:, 2 * pr:2 * pr + 2, h0:h1],
                            start=(pr == 0), stop=(pr == KF // 2 - 1),
                            perf_mode=mybir.MatmulPerfMode.DoubleRow)
                # t1 = sw2*(x+b2) + psU  (scale-invariant LN1 input);
                # releases psU as soon as it runs
                t1 = wbf.tile([P, H], BF16, tag="t1")
                nc.vector.scalar_tensor_tensor(
                    t1, xrsb[:, c * TPT + m], s2t, psU,
                    op0=mybir.AluOpType.mult, op1=mybir.AluOpType.add,
                    accum_out=tsum1[:, m:m + 1])
                sumsq(t1, s2s1[:, m:m + 1])
                t1s.append(t1)
            mean1, vv1 = var_chain(tsum1, s2s1)
            rs1 = rsqrt_batch(vv1)
            if pend_a is not None:
                stage_b2(*stage_b1(*pend_a))
            pend_a = (c, t1s, mean1, rs1)
            for tiles in x0_at.get(c, ()):
                x0_batch(tiles)
        if pend_a is not None:
            stage_b2(*stage_b1(*pend_a))

    nc.finalize()
    return nc


_NC_CACHE: dict[tuple, bass.Bass] = {}


def get_nc(cap: int, cap0: int, repeat: int = 1,
           flags: tuple = (True, True, True, True)) -> bass.Bass:
    key = (cap, cap0, repeat, flags)
    if key not in _NC_CACHE:
        b1z, b2z, lnt, outt = flags
        _NC_CACHE[key] = build_nc(cap, cap0, repeat, b1_zero=b1z, b2_zero=b2z,
                                  ln_trivial=lnt, out_trivial=outt)
    return _NC_CACHE[key]


def _round_up(n: int, m: int) -> int:
    return max(m, ((n + m - 1) // m) * m)


def shard_inputs(input_tensor, type_seq, W1, b1, W2, b2, ln_g, ln_b, out_g, out_b):
    """Host-side routing + fp8/bf16 prep. Returns (in_maps, core_tokens,
    zero_splits, cap, cap0, flags)."""
    B, L, _H = input_tensor.shape
    assert _H == H, f"kernel hardcodes d_model={H}, got {_H}"
    x = np.ascontiguousarray(np.asarray(input_tensor, dtype=np.float32)).reshape(B * L, H)
    ts_flat = np.asarray(type_seq).reshape(-1).astype(np.int64)
    NB = W1.shape[0]
    per_expert = max(1, NCORES // NB)
    W1 = np.asarray(W1, dtype=np.float32)
    W2 = np.asarray(W2, dtype=np.float32)
    b1 = np.asarray(b1, dtype=np.float32)
    b2 = np.asarray(b2, dtype=np.float32)

    flags = (
        not b1.any(),
        not b2.any(),
        bool(np.all(ln_g == 1.0) and not np.asarray(ln_b).any()),
        bool(np.all(out_g == 1.0) and not np.asarray(out_b).any()),
    )

    core_tokens = []
    core_expert = []
    for e in range(NB):
        toks = np.nonzero(ts_flat == e + 1)[0]
        for s in np.array_split(toks, per_expert):
            core_tokens.append(s)
            core_expert.append(e)
    while len(core_tokens) < NCORES:
        core_tokens.append(np.zeros(0, dtype=np.int64))
        core_expert.append(0)
    zero_splits = np.array_split(np.nonzero(ts_flat == 0)[0], NCORES)

    cap = _round_up(max(len(t) for t in core_tokens), TCH)
    cap0 = _round_up(max(len(z) for z in zero_splits), P)

    sx = _pow2_scale(float(np.abs(x).max()))
    xq = (x * np.float32(sx)).astype(E4M3)  # global; sliced per core

    sw1 = [_pow2_scale(float(np.abs(W1[e]).max())) for e in range(NB)]
    sw2 = [_pow2_scale(float(np.abs(W2[e]).max())) for e in range(NB)]
    w1q = [(W1[e] * np.float32(sw1[e])).astype(E4M3) for e in range(NB)]
    w2q = [(W2[e] * np.float32(sw2[e])).astype(E4M3) for e in range(NB)]

    def f32c(a):
        return np.ascontiguousarray(np.asarray(a, dtype=np.float32))

    in_maps = []
    for cidx in range(NCORES):
        toks = core_tokens[cidx]
        e = core_expert[cidx]
        z = zero_splits[cidx]
        xrT = np.zeros((H, cap), E4M3)
        xrT[:, : len(toks)] = xq[toks].T
        xr = np.zeros((cap, H), NP_BF16)
        xr[: len(toks)] = (x[toks] + b2[e]).astype(NP_BF16)
        x0 = np.zeros((cap0, H), NP_BF16)
        x0[: len(z)] = x[z].astype(NP_BF16)
        im = {
            "xrT": np.ascontiguousarray(xrT),
            "w1": w1q[e],
            "w2": w2q[e],
            "xr": xr,
            "x0": x0,
            "ginv1": np.array([1.0 / (sx * sw1[e])], np.float32),
            "s2": np.array([sw2[e]], np.float32),
        }
        if not flags[0]:
            im["b1"] = f32c(b1[e])
        if not flags[1]:
            xr0 = np.zeros((cap, H), NP_BF16)
            xr0[: len(toks)] = x[toks].astype(NP_BF16)
            im["xr0"] = xr0
        if not flags[2]:
            im["lng"] = f32c(ln_g[e])
            im["lnb"] = f32c(ln_b[e])
        if not flags[3]:
            im["outg"] = f32c(out_g)
            im["outb"] = f32c(out_b)
        in_maps.append(im)
    return in_maps, core_tokens, zero_splits, cap, cap0, flags


def unshard_output(results, core_tokens, zero_splits, shape, dtype):
    B, L, _H = shape
    out = np.empty((B * L, H), np.float32)
    for c in range(NCORES):
        toks = core_tokens[c]
        z = zero_splits[c]
        if len(toks):
            out[toks] = results[c]["yr"][: len(toks)]
        if len(z):
            out[z] = results[c]["y0"][: len(z)]
    return out.reshape(B, L, H).astype(dtype, copy=False)


def kernel(input_tensor, type_seq, W1, b1, W2, b2, ln_g, ln_b, out_g, out_b):
    in_maps, core_tokens, zero_splits, cap, cap0, flags = shard_inputs(
        input_tensor, type_seq, W1, b1, W2, b2, ln_g, ln_b, out_g, out_b
    )
    nc = get_nc(cap, cap0, flags=flags)
    res = run_bass_kernel_spmd(nc, in_maps, core_ids=list(range(NCORES)))
    return unshard_output(
        res.results, core_tokens, zero_splits, input_tensor.shape,
        np.asarray(input_tensor).dtype,
    )


# revision 32
# speedup vs baseline: 1.0889x; 1.0303x over previous
"""Trainium2 Bass kernel for nn_MBSFeedForward (moe_routing) — fp8 edition.

Reference semantics (per token t with class c = type_seq[t]):
  c == 0:  out = LN_out(x_t)
  c >= 1:  e = c-1 (expert)
           u = GELU(x_t @ W1_e + b1_e) @ W2_e + b2_e
           v = LN_e(u + x_t)          (per-expert ln_g/ln_b)
           out = LN_out(v + x_t)      (out_g/out_b)

Sharding (host-side routing): 4 experts x 2 cores each; class-0 tokens
split over all 8 cores (outer LN only). No collectives; host scatters.

Device kernel design:
  - Both GEMMs run as fp8(e4m3) DoubleRow matmuls: each instruction
    contracts TWO 128-deep k-tiles (operands laid out [128, 2, n]) at
    0.5 PE cycles per output row. Host pre-quantizes x/W1/W2 with
    power-of-2 scales (RNE via ml_dtypes); the only on-device fp8
    rounding is the gelu->fp8 write, which the scalar engine does RNE
    (verified on HW). End-to-end rel err ~1.8e-2 vs the 2e-2 gate.
  - GEMM2 result arrives in PSUM scaled by sw2; LN is scale-invariant,
    so the first residual add computes sw2*(x+b2) + psU in one DVE
    scalar_tensor_tensor (sw2 rides along as a [P,1] scalar input) and
    the normalize needs no unscale. b2 folds into the residual array.
  - LayerNorm: bn_stats/bn_aggr on DVE, rsqrt via bit-trick + Newton on
    GPSIMD (keeps Sqrt off the scalar engine so the Gelu activation
    table is loaded exactly once), LN1 normalize on DVE (bf16 2x), the
    final normalize on GPSIMD writing f32.
  - All inputs are DMA'd to SBUF once, on the two HWDGE queues
    (SP/Activation) in the order chunk 0 consumes them; steady state
    does only output-store DMAs.
"""

import math
from contextlib import ExitStack

import numpy as np
import ml_dtypes

import concourse.bass as bass
import concourse.tile as tile
from concourse import bacc
from concourse import mybir
from concourse.bass_utils import run_bass_kernel_spmd

F32 = mybir.dt.float32
BF16 = mybir.dt.bfloat16
FP8 = mybir.dt.float8e4
I32 = mybir.dt.int32
E4M3 = ml_dtypes.float8_e4m3
NP_BF16 = ml_dtypes.bfloat16

P = 128
H = 768
F = 3072
KH = H // P     # 6  (k-tiles for GEMM1)
KF = F // P     # 24 (k-tiles for GEMM2)
TCH = 256       # tokens per chunk
TPT = TCH // P  # token tiles per chunk (2)
QK = 4          # kf-tiles per ps1/gelu batch
NQ = KF // QK   # 6 ps1 batches per chunk
NCORES = 8
RSQRT_MAGIC = 0x5F3759DF
FP8_TARGET = 192.0  # max|v*scale| target; e4m3 (IEEE) max finite is 240
NEWTON2 = True      # second Newton step for rsqrt

ACT_FUNC = mybir.ActivationFunctionType.Gelu


def _pow2_scale(m: float) -> float:
    if m <= 0 or not math.isfinite(m):
        return 1.0
    return 2.0 ** math.floor(math.log2(FP8_TARGET / m))


def build_nc(cap: int, cap0: int, repeat: int = 1, *, b1_zero: bool = True,
             b2_zero: bool = True, ln_trivial: bool = True,
             out_trivial: bool = True) -> bass.Bass:
    """Per-core Bass module for `cap` routed tokens and `cap0` LN-only
    tokens. repeat>1 re-runs the body (slope timing)."""
    assert cap % TCH == 0 and cap0 % P == 0
    nc = bacc.Bacc()
    chunks = cap // TCH
    nt0 = cap0 // P
    ntile = cap // P

    d_xrT = nc.dram_tensor("xrT", [H, cap], FP8, kind="ExternalInput")
    d_w1 = nc.dram_tensor("w1", [H, F], FP8, kind="ExternalInput")
    d_w2 = nc.dram_tensor("w2", [F, H], FP8, kind="ExternalInput")
    d_xr = nc.dram_tensor("xr", [cap, H], BF16, kind="ExternalInput")
    d_x0 = nc.dram_tensor("x0", [cap0, H], BF16, kind="ExternalInput")
    d_ginv1 = nc.dram_tensor("ginv1", [1], F32, kind="ExternalInput")
    d_s2 = nc.dram_tensor("s2", [1], F32, kind="ExternalInput")
    d_b1 = None if b1_zero else nc.dram_tensor("b1", [F], F32, kind="ExternalInput")
    # when b2 != 0, the residual-2 array (plain x) is separate from xr (x+b2)
    d_xr0 = None if b2_zero else nc.dram_tensor("xr0", [cap, H], BF16, kind="ExternalInput")
    d_lng = d_lnb = d_outg = d_outb = None
    if not ln_trivial:
        d_lng = nc.dram_tensor("lng", [H], F32, kind="ExternalInput")
        d_lnb = nc.dram_tensor("lnb", [H], F32, kind="ExternalInput")
    if not out_trivial:
        d_outg = nc.dram_tensor("outg", [H], F32, kind="ExternalInput")
        d_outb = nc.dram_tensor("outb", [H], F32, kind="ExternalInput")
    d_yr = nc.dram_tensor("yr", [cap, H], F32, kind="ExternalOutput")
    d_y0 = nc.dram_tensor("y0", [cap0, H], F32, kind="ExternalOutput")

    with tile.TileContext(nc) as tc, ExitStack() as ctx:
        singles = ctx.enter_context(tc.tile_pool(name="singles", bufs=1))
        hpool = ctx.enter_context(tc.tile_pool(name="hact", bufs=20))
        wbf = ctx.enter_context(tc.tile_pool(name="wbf", bufs=12))
        wf32 = ctx.enter_context(tc.tile_pool(name="wf32", bufs=8))
        stat = ctx.enter_context(tc.tile_pool(name="stat", bufs=48))
        ps1p = ctx.enter_context(tc.tile_pool(name="ps1", bufs=2, space="PSUM"))
        psUp = ctx.enter_context(tc.tile_pool(name="psU", bufs=2, space="PSUM"))

        # --- resident inputs on the two HWDGE queues, in the order chunk 0
        # consumes them: xrT c0, w1 (GEMM1), w2 (GEMM2), rest of xrT ---
        w1sb = singles.tile([P, KH, F], FP8)
        w1_v = d_w1[:].rearrange("(ko ki) f -> ki ko f", ki=P)
        w2sb = singles.tile([P, KF, H], FP8)
        w2_v = d_w2[:].rearrange("(ko ki) h -> ki ko h", ki=P)
        xrTsb = singles.tile([P, KH, cap], FP8)
        xrT_v = d_xrT[:].rearrange("(ko ki) t -> ki ko t", ki=P)
        c0 = min(3 * TCH, cap)
        nc.sync.dma_start(xrTsb[:, :, 0:c0], xrT_v[:, :, 0:c0])
        # interleave w1 thirds with w2 sixths so chunk 0's GEMM2 operands
        # land while its GEMM1 still runs
        for s in range(3):
            sl = slice(s * F // 3, (s + 1) * F // 3)
            nc.scalar.dma_start(w1sb[:, :, sl], w1_v[:, :, sl])
            for t in (2 * s, 2 * s + 1):
                tl = slice(t * KF // 6, (t + 1) * KF // 6)
                nc.scalar.dma_start(w2sb[:, tl], w2_v[:, tl])
        if c0 < cap:
            nc.sync.dma_start(xrTsb[:, :, c0:cap], xrT_v[:, :, c0:cap])

        x0sb = singles.tile([P, max(nt0, 1), H], BF16)
        if nt0:
            nc.sync.dma_start(x0sb[:, 0:nt0], d_x0[:].rearrange("(n p) h -> p n h", p=P))
        xrsb = singles.tile([P, ntile, H], BF16)
        xr_v = d_xr[:].rearrange("(n p) h -> p n h", p=P)
        nc.sync.dma_start(xrsb[:, 0:2], xr_v[:, 0:2])
        nc.sync.dma_start(xrsb[:, 2:ntile // 2], xr_v[:, 2:ntile // 2])
        nc.sync.dma_start(xrsb[:, ntile // 2:], xr_v[:, ntile // 2:])
        xr0sb = xrsb
        if not b2_zero:
            xr0sb = singles.tile([P, ntile, H], BF16, name="xr0sb")
            nc.sync.dma_start(xr0sb, d_xr0[:].rearrange("(n p) h -> p n h", p=P))

        ginv1 = singles.tile([P, 1], F32, name="ginv1")
        nc.gpsimd.dma_start(ginv1, d_ginv1[:][None, :].to_broadcast([P, 1]))
        s2t = singles.tile([P, 1], F32, name="s2t")
        nc.gpsimd.dma_start(s2t, d_s2[:][None, :].to_broadcast([P, 1]))

        def bc_tile(d, nm):
            t = singles.tile([P, H], F32, tag=nm, name=nm)
            nc.gpsimd.dma_start(t, d[:][None, :].to_broadcast([P, H]))
            return t

        lngbc = lnbbc = outgbc = outbbc = None
        if not ln_trivial:
            lngbc, lnbbc = bc_tile(d_lng, "lngbc"), bc_tile(d_lnb, "lnbbc")
        if not out_trivial:
            outgbc, outbbc = bc_tile(d_outg, "outgbc"), bc_tile(d_outb, "outbbc")
        b1sb = None
        if not b1_zero:
            b1sb = singles.tile([P, KF], F32, name="b1sb")
            nc.gpsimd.dma_start(b1sb, d_b1[:].rearrange("(o p) -> p o", p=P))

        def sumsq(src, acc):
            """DVE: acc[P,1] = sum(src^2)/H via tensor_tensor_reduce (bf16
            operands keep the 2x DVE mode; the product tensor goes to a
            write-only scratch)."""
            sq = wbf.tile([P, H], BF16, tag="sq")
            nc.vector.tensor_tensor_reduce(
                sq, src, src, 1.0 / H, 0.0,
                op0=mybir.AluOpType.mult, op1=mybir.AluOpType.add,
                accum_out=acc)

        def var_chain(tsum, s2s):
            """Pool: mean = tsum/H, var = s2s - mean^2 for [P,k] batches.
            Means here are << std (residual streams are near zero-mean per
            token), so the subtraction loses no precision."""
            k = tsum.shape[-1]
            mean = stat.tile([P, k], F32, tag="mean")
            nc.gpsimd.tensor_scalar(mean, tsum, scalar1=1.0 / H, scalar2=None,
                                    op0=mybir.AluOpType.mult)
            z = stat.tile([P, k], F32, tag="zm")
            nc.gpsimd.tensor_tensor(z, mean, mean, op=mybir.AluOpType.mult)
            vv = stat.tile([P, k], F32, tag="vv")
            nc.gpsimd.tensor_tensor(vv, s2s, z, op=mybir.AluOpType.subtract)
            return mean, vv

        def rsqrt_batch(v):
            """rs = 1/sqrt(v) elementwise for v [P, k] (k small). Quake seed
            y0 = float_bits(magic - (v_int >> 1)) built as (~(v>>1)) +
            (magic+1) — shift+xor fuse (bitwise pair) and there is no
            reversed subtract; GPSIMD fails the ISA check for shifts so the
            seed runs on DVE. Newton steps y' = (1.5 - 0.5*v*y^2)*y go on
            GPSIMD as stt/tt/stt with signs folded into the constants."""
            k = v.shape[-1]
            yi = stat.tile([P, k], I32, tag="yi")
            nc.vector.tensor_scalar(
                yi, v.bitcast(I32), scalar1=1, scalar2=-1,
                op0=mybir.AluOpType.logical_shift_right,
                op1=mybir.AluOpType.bitwise_xor)
            nc.vector.tensor_scalar(
                yi, yi, scalar1=RSQRT_MAGIC + 1, scalar2=None,
                op0=mybir.AluOpType.add)
            rs = yi.bitcast(F32)
            for _ in range(1 + int(NEWTON2)):
                z = stat.tile([P, k], F32, tag="z")
                nc.gpsimd.tensor_tensor(z, rs, rs, op=mybir.AluOpType.mult)
                nc.gpsimd.tensor_tensor(z, z, v, op=mybir.AluOpType.mult)
                nc.gpsimd.tensor_scalar(
                    z, z, scalar1=-0.5, scalar2=-1.5,
                    op0=mybir.AluOpType.mult, op1=mybir.AluOpType.subtract)
                rs_n = stat.tile([P, k], F32, tag="rsn")
                nc.gpsimd.tensor_tensor(rs_n, z, rs, op=mybir.AluOpType.mult)
                rs = rs_n
            return rs

        def normalize(src, dst, mean, rs, eng, gbc, bbc):
            eng.tensor_scalar(
                dst, src, scalar1=mean, scalar2=rs,
                op0=mybir.AluOpType.subtract, op1=mybir.AluOpType.mult)
            if gbc is not None:
                nc.gpsimd.tensor_mul(dst, dst, gbc)
                nc.gpsimd.tensor_add(dst, dst, bbc)

        def neg_m_rs(mv, rs):
            """[P,k] bias tile -mean*rs for activation-engine normalizes:
            activation(Identity, scale=rs, bias=-m*rs) == (t - m) * rs."""
            k = rs.shape[-1]
            nrs = stat.tile([P, k], F32, tag="nrs")
            nc.gpsimd.tensor_scalar(nrs, rs, scalar1=-1.0, scalar2=None,
                                    op0=mybir.AluOpType.mult)
            mb = stat.tile([P, k], F32, tag="mb")
            nc.gpsimd.tensor_tensor(mb, mv[:, :, 0], nrs,
                                    op=mybir.AluOpType.mult)
            return mb

        def act_normalize(src, dst, mb, rs, gbc, bbc):
            nc.scalar.activation(
                dst, src, mybir.ActivationFunctionType.Identity,
                bias=mb, scale=rs)
            if gbc is not None:
                nc.gpsimd.tensor_mul(dst, dst, gbc)
                nc.gpsimd.tensor_add(dst, dst, bbc)

        def x0_batch(tiles):
            """LN-only tiles, pair-batched rsqrt; normalizes on GPSIMD
            (keeps the gelu-feeding scalar queue free of waits)."""
            k = len(tiles)
            tsum = stat.tile([P, k], F32, tag="tsum0")
            s2s = stat.tile([P, k], F32, tag="s2s0")
            for i, n in enumerate(tiles):
                nc.vector.tensor_reduce(
                    tsum[:, i:i + 1], x0sb[:, n], mybir.AxisListType.X,
                    mybir.AluOpType.add)
                sumsq(x0sb[:, n], s2s[:, i:i + 1])
            mean, vv = var_chain(tsum, s2s)
            rs = rsqrt_batch(vv)
            for i, n in enumerate(tiles):
                o = wf32.tile([P, H], F32, tag="o")
                normalize(x0sb[:, n], o, mean[:, i:i + 1], rs[:, i:i + 1],
                          nc.gpsimd, outgbc, outbbc)
                nc.sync.dma_start(d_y0[n * P:(n + 1) * P], o)

        # spread LN-only batches across mid-loop chunks (away from the ramp)
        x0_at = {}
        for b in range((nt0 + 1) // 2):
            tiles = list(range(2 * b, min(2 * b + 2, nt0)))
            x0_at.setdefault(min(b, chunks - 1), []).append(tiles)

        def stage_b1(c, t1s, mean1, rs1):
            """norm1 -> +x residual -> LN2 stats/rsqrt for chunk c (one
            chunk behind stage A so every op's inputs are computed —
            avoids head-of-line stalls on the in-order queues)."""
            t2s = []
            tsum2 = stat.tile([P, TPT], F32, tag="tsum2")
            s2s2 = stat.tile([P, TPT], F32, tag="s2s2")
            for m in range(TPT):
                v1 = wbf.tile([P, H], BF16, tag="v1")
                normalize(t1s[m], v1, mean1[:, m:m + 1], rs1[:, m:m + 1],
                          nc.vector if m == 0 else nc.gpsimd, lngbc, lnbbc)
                t2 = wbf.tile([P, H], BF16, tag="t2")
                nc.vector.scalar_tensor_tensor(
                    t2, v1, 1.0, xr0sb[:, c * TPT + m],
                    op0=mybir.AluOpType.mult, op1=mybir.AluOpType.add,
                    accum_out=tsum2[:, m:m + 1])
                sumsq(t2, s2s2[:, m:m + 1])
                t2s.append(t2)
            mean2, vv2 = var_chain(tsum2, s2s2)
            rs2 = rsqrt_batch(vv2)
            return (c, t2s, mean2, rs2)

        def stage_b2(c, t2s, mean2, rs2):
            """final normalizes + stores for chunk c."""
            for m in range(TPT):
                o = wf32.tile([P, H], F32, tag="o")
                normalize(t2s[m], o, mean2[:, m:m + 1], rs2[:, m:m + 1],
                          nc.gpsimd, outgbc, outbbc)
                n = c * TPT + m
                nc.sync.dma_start(d_yr[n * P:(n + 1) * P], o)

        def stage_a1(c):
            """GEMM1 + gelu: 6 batches of 4 kf-tiles. Runs one chunk ahead
            of stage_a2 so chunk 0's w2-waiting GEMM2 instructions don't
            head-of-line block later chunks' GEMM1 on the in-order PE
            queue during the load ramp."""
            has = []
            for q in range(NQ):
                ps1 = ps1p.tile([P, QK, TCH], F32, tag="ps1")
                for j in range(QK):
                    kf = q * QK + j
                    for t in range(KH // 2):
                        nc.tensor.matmul(
                            ps1[:, j],
                            w1sb[:, 2 * t:2 * t + 2, kf * P:(kf + 1) * P],
                            xrTsb[:, 2 * t:2 * t + 2, c * TCH:(c + 1) * TCH],
                            start=(t == 0), stop=(t == KH // 2 - 1),
                            perf_mode=mybir.MatmulPerfMode.DoubleRow)
                ha = hpool.tile([P, QK, TCH], FP8, tag="ha")
                if b1_zero:
                    nc.scalar.activation(ha, ps1, ACT_FUNC, bias=0.0, scale=ginv1)
                else:
                    for j in range(QK):
                        kf = q * QK + j
                        nc.scalar.activation(
                            ha[:, j], ps1[:, j], ACT_FUNC,
                            bias=b1sb[:, kf:kf + 1], scale=ginv1)
                has.append(ha)
            return (c, has)

        def stage_a2(c, has):
            """GEMM2 per 128-token tile; LN1 stats pair-batched."""
            t1s = []
            tsum1 = stat.tile([P, TPT], F32, tag="tsum1")
            s2s1 = stat.tile([P, TPT], F32, tag="s2s1")
            for m in range(TPT):
                psU = psUp.tile([P, H], F32, tag="psU")
                for pr in range(KF // 2):
                    q, b = divmod(pr, QK // 2)
                    lhsT = has[q][:, 2 * b:2 * b + 2, m * P:(m + 1) * P]
                    for h0, h1 in ((0, 512), (512, H)):
                        nc.tensor.matmul(
                            psU[:, h0:h1], lhsT,
                            w2sb[:, 2 * pr:2 * pr + 2, h0:h1],
                            start=(pr == 0), stop=(pr == KF // 2 - 1),
                            perf_mode=mybir.MatmulPerfMode.DoubleRow)
                # t1 = sw2*(x+b2) + psU  (scale-invariant LN1 input);
                # releases psU as soon as it runs
                t1 = wbf.tile([P, H], BF16, tag="t1")
                nc.vector.scalar_tensor_tensor(
                    t1, xrsb[:, c * TPT + m], s2t, psU,
                    op0=mybir.AluOpType.mult, op1=mybir.AluOpType.add,
                    accum_out=tsum1[:, m:m + 1])
                sumsq(t1, s2s1[:, m:m + 1])
                t1s.append(t1)
            mean1, vv1 = var_chain(tsum1, s2s1)
            rs1 = rsqrt_batch(vv1)
            return (c, t1s, mean1, rs1)

        pend_a1 = None  # awaiting stage_a2 (GEMM2)
        pend_a2 = None  # awaiting stage_b1+b2 (LN chain)
        for it in range(repeat * chunks):
            c = it % chunks
            a1 = stage_a1(c)
            if pend_a1 is not None:
                next_a2 = stage_a2(*pend_a1)
            else:
                next_a2 = None
            pend_a1 = a1
            if pend_a2 is not None:
                stage_b2(*stage_b1(*pend_a2))
            pend_a2 = next_a2
            for tiles in x0_at.get(c, ()):
                x0_batch(tiles)
        if pend_a1 is not None:
            next_a2 = stage_a2(*pend_a1)
            if pend_a2 is not None:
                stage_b2(*stage_b1(*pend_a2))
            stage_b2(*stage_b1(*next_a2))

    nc.finalize()
    return nc


_NC_CACHE: dict[tuple, bass.Bass] = {}


def get_nc(cap: int, cap0: int, repeat: int = 1,
           flags: tuple = (True, True, True, True)) -> bass.Bass:
    key = (cap, cap0, repeat, flags)
    if key not in _NC_CACHE:
        b1z, b2z, lnt, outt = flags
        _NC_CACHE[key] = build_nc(cap, cap0, repeat, b1_zero=b1z, b2_zero=b2z,
                                  ln_trivial=lnt, out_trivial=outt)
    return _NC_CACHE[key]


def _round_up(n: int, m: int) -> int:
    return max(m, ((n + m - 1) // m) * m)


def shard_inputs(input_tensor, type_seq, W1, b1, W2, b2, ln_g, ln_b, out_g, out_b):
    """Host-side routing + fp8/bf16 prep. Returns (in_maps, core_tokens,
    zero_splits, cap, cap0, flags)."""
    B, L, _H = input_tensor.shape
    assert _H == H, f"kernel hardcodes d_model={H}, got {_H}"
    x = np.ascontiguousarray(np.asarray(input_tensor, dtype=np.float32)).reshape(B * L, H)
    ts_flat = np.asarray(type_seq).reshape(-1).astype(np.int64)
    NB = W1.shape[0]
    per_expert = max(1, NCORES // NB)
    W1 = np.asarray(W1, dtype=np.float32)
    W2 = np.asarray(W2, dtype=np.float32)
    b1 = np.asarray(b1, dtype=np.float32)
    b2 = np.asarray(b2, dtype=np.float32)

    flags = (
        not b1.any(),
        not b2.any(),
        bool(np.all(ln_g == 1.0) and not np.asarray(ln_b).any()),
        bool(np.all(out_g == 1.0) and not np.asarray(out_b).any()),
    )

    core_tokens = []
    core_expert = []
    for e in range(NB):
        toks = np.nonzero(ts_flat == e + 1)[0]
        for s in np.array_split(toks, per_expert):
            core_tokens.append(s)
            core_expert.append(e)
    while len(core_tokens) < NCORES:
        core_tokens.append(np.zeros(0, dtype=np.int64))
        core_expert.append(0)
    zero_splits = np.array_split(np.nonzero(ts_flat == 0)[0], NCORES)

    cap = _round_up(max(len(t) for t in core_tokens), TCH)
    cap0 = _round_up(max(len(z) for z in zero_splits), P)

    sx = _pow2_scale(float(np.abs(x).max()))
    xq = (x * np.float32(sx)).astype(E4M3)  # global; sliced per core

    sw1 = [_pow2_scale(float(np.abs(W1[e]).max())) for e in range(NB)]
    sw2 = [_pow2_scale(float(np.abs(W2[e]).max())) for e in range(NB)]
    w1q = [(W1[e] * np.float32(sw1[e])).astype(E4M3) for e in range(NB)]
    w2q = [(W2[e] * np.float32(sw2[e])).astype(E4M3) for e in range(NB)]

    def f32c(a):
        return np.ascontiguousarray(np.asarray(a, dtype=np.float32))

    in_maps = []
    for cidx in range(NCORES):
        toks = core_tokens[cidx]
        e = core_expert[cidx]
        z = zero_splits[cidx]
        xrT = np.zeros((H, cap), E4M3)
        xrT[:, : len(toks)] = xq[toks].T
        xr = np.zeros((cap, H), NP_BF16)
        xr[: len(toks)] = (x[toks] + b2[e]).astype(NP_BF16)
        x0 = np.zeros((cap0, H), NP_BF16)
        x0[: len(z)] = x[z].astype(NP_BF16)
        im = {
            "xrT": np.ascontiguousarray(xrT),
            "w1": w1q[e],
            "w2": w2q[e],
            "xr": xr,
            "x0": x0,
            "ginv1": np.array([1.0 / (sx * sw1[e])], np.float32),
            "s2": np.array([sw2[e]], np.float32),
        }
        if not flags[0]:
            im["b1"] = f32c(b1[e])
        if not flags[1]:
            xr0 = np.zeros((cap, H), NP_BF16)
            xr0[: len(toks)] = x[toks].astype(NP_BF16)
            im["xr0"] = xr0
        if not flags[2]:
            im["lng"] = f32c(ln_g[e])
            im["lnb"] = f32c(ln_b[e])
        if not flags[3]:
            im["outg"] = f32c(out_g)
            im["outb"] = f32c(out_b)
        in_maps.append(im)
    return in_maps, core_tokens, zero_splits, cap, cap0, flags


def unshard_output(results, core_tokens, zero_splits, shape, dtype):
    B, L, _H = shape
    out = np.empty((B * L, H), np.float32)
    for c in range(NCORES):
        toks = core_tokens[c]
        z = zero_splits[c]
        if len(toks):
            out[toks] = results[c]["yr"][: len(toks)]
        if len(z):
            out[z] = results[c]["y0"][: len(z)]
    return out.reshape(B, L, H).astype(dtype, copy=False)


def kernel(input_tensor, type_seq, W1, b1, W2, b2, ln_g, ln_b, out_g, out_b):
    in_maps, core_tokens, zero_splits, cap, cap0, flags = shard_inputs(
        input_tensor, type_seq, W1, b1, W2, b2, ln_g, ln_b, out_g, out_b
    )
    nc = get_nc(cap, cap0, flags=flags)
    res = run_bass_kernel_spmd(nc, in_maps, core_ids=list(range(NCORES)))
    return unshard_output(
        res.results, core_tokens, zero_splits, input_tensor.shape,
        np.asarray(input_tensor).dtype,
    )
